# revision 29
# baseline (speedup 1.0000x reference)
"""Additive fast-weight (linear attention) layer on 8 TRN2 NeuronCores.

Strategy:
  - Shard batch (B=8) across the 8 cores: the recurrent state and scan are
    independent per (b, h); slow-net / out-linear weights are replicated.
  - Per core: LayerNorm -> qkv matmul -> elu+1 feature map with sum-norm ->
    chunked-parallel linear attention (chunk C=128 converts the 2048-step
    scan into matmuls: O = Q.W0 + tril(Q K^T).V ; W0 += K^T V) -> out matmul
    -> residual.
  - Matmuls run in bf16 with fp32 PSUM accumulation; the recurrent state is
    accumulated in fp32 in SBUF.  LayerNorm gamma is folded into W_qkv and
    beta into a per-feature bias (added via a K=1 matmul) on the host.
"""

import os
import numpy as np
import ml_dtypes

import concourse.bass as bass
import concourse.mybir as mybir
import concourse.tile as tile
from concourse.bass_utils import run_bass_kernel_spmd
from concourse.masks import make_identity

F32 = mybir.dt.float32
BF16 = mybir.dt.bfloat16
AF = mybir.ActivationFunctionType
ALU = mybir.AluOpType

NUM_HEAD, DIM_HEAD, IN_DIM = 16, 64, 1024
SLEN, BSZ = 2048, 8
C = 128  # chunk length (tokens per chunk)
NPAIR = NUM_HEAD // 2  # head pairs (2 heads share a 128-partition tile)


# Max semaphore waits each ISA struct can encode (observed from walrus
# codegen failures / successes).  Excess waits are moved onto standalone
# EventSemaphore instructions on the same engine right before, which is
# sound for compute engines (sequencer executes in program order).
_WAIT_CAPS = {}
_DMA_TYPES = {"InstDMACopy", "InstDmaTransposeAnt"}
_NO_SPLIT = {
    "InstEventSemaphore", "InstCall", "InstUnconditionalBranch",
    "InstRegisterMove", "InstISA",
}


def _split_excess_waits(nc):
    """Walrus can encode only one sem-wait per instruction (per ISA struct).

    Compute engines: move excess waits onto standalone EventSemaphore
    instructions on the same engine right before (program order makes this
    sound).  DMA descriptors: the wait fires the descriptor, so instead a
    chain of EventSemaphores on SP absorbs ALL original waits and then
    increments a fresh one-shot semaphore that the DMA's single slot waits on.
    """
    used_ids = set()
    for f in nc.m.functions:
        for blk in f.blocks:
            for inst in blk.instructions:
                si = inst.sync_info
                if si is not None:
                    for w in (si.on_wait or []):
                        if w.sync_type == 'semaphore':
                            used_ids.add(w.id)
                    for u in (si.on_update or []):
                        if u.sync_type == 'semaphore':
                            used_ids.add(u.id)
    free_ids = sorted([i for i in range(256) if i not in used_ids], reverse=True)  # pop() -> lowest first
    sink_id = free_ids.pop()

    def _sink_upd():
        return [mybir.SyncUpdate(sync_type='semaphore', id=sink_id,
                                 ant_name='wsink', update_mode='sem-inc',
                                 update_value=1, update_reg=None)]

    n_split = 0
    n_join = 0
    for f in nc.m.functions:
        for blk in f.blocks:
            out = []
            changed = False
            for inst in blk.instructions:
                si = inst.sync_info
                tn = type(inst).__name__
                if si is not None and si.on_wait and tn not in _NO_SPLIT:
                    waits = list(si.on_wait)
                    cap = _WAIT_CAPS.get(tn, 1)
                    is_dma = tn in _DMA_TYPES and inst.engine != mybir.EngineType.Pool
                    if is_dma and len(waits) > 1 and os.environ.get('BISECT_DROP'):
                        inst.sync_info = mybir.SyncInfo(
                            on_wait=[waits[0]], on_update=list(si.on_update))
                        changed = True
                    elif is_dma and len(waits) > 1 and not os.environ.get('NO_JOIN'):
                        sem_id = free_ids.pop()
                        sem_name = f"wjoin{n_join}"
                        n_join += 1
                        for i, w in enumerate(waits):
                            ev = mybir.InstEventSemaphore(
                                name=f"I-wjoin-{n_join}-{i}", ins=[], outs=[])
                            ev.engine = mybir.EngineType.Pool
                            upd = _sink_upd()
                            if i == len(waits) - 1:
                                upd = [mybir.SyncUpdate(
                                    sync_type='semaphore', id=sem_id,
                                    ant_name=sem_name, update_mode='sem-inc',
                                    update_value=1, update_reg=None)]
                            ev.sync_info = mybir.SyncInfo(on_wait=[w], on_update=upd)
                            out.append(ev)
                        inst.sync_info = mybir.SyncInfo(
                            on_wait=[mybir.SyncWait(
                                sync_type='semaphore', id=sem_id,
                                ant_name=sem_name, wait_mode='sem-ge-imm',
                                wait_value=1, wait_reg=None)],
                            on_update=list(si.on_update))
                        changed = True
                    elif not is_dma and len(waits) > cap:
                        for w in waits[cap:]:
                            ev = mybir.InstEventSemaphore(
                                name=f"I-wsplit-{n_split}", ins=[], outs=[])
                            n_split += 1
                            ev.engine = inst.engine
                            ev.sync_info = mybir.SyncInfo(on_wait=[w], on_update=_sink_upd())
                            out.append(ev)
                        inst.sync_info = mybir.SyncInfo(
                            on_wait=waits[:cap], on_update=list(si.on_update))
                        changed = True
                out.append(inst)
            if changed:
                blk.instructions = out
    if n_join or n_split:
        reg = dict(nc.m.ant_sem_names)
        reg[str(sink_id)] = ['wsink']
        for f in nc.m.functions:
            for blk in f.blocks:
                for inst in blk.instructions:
                    si = inst.sync_info
                    if si is not None:
                        for u in (si.on_update or []):
                            if u.ant_name.startswith("wjoin"):
                                reg[str(u.id)] = [u.ant_name]
        nc.m.ant_sem_names = reg
    return n_split + n_join


def build_nc(n_chunks: int = SLEN // C, split_waits: bool = True) -> bass.Bass:
    S = n_chunks * C
    nc = bass.Bass()

    x_in = nc.declare_dram_parameter("x", [S, IN_DIM], F32, isOutput=False)
    st_in = nc.declare_dram_parameter("st", [128, NPAIR, 128], F32, isOutput=False)
    wq_in = nc.declare_dram_parameter("wq", [IN_DIM, IN_DIM], BF16, isOutput=False)
    wk_in = nc.declare_dram_parameter("wk", [IN_DIM, IN_DIM], BF16, isOutput=False)
    wv_in = nc.declare_dram_parameter("wv", [IN_DIM, IN_DIM], BF16, isOutput=False)
    wo_in = nc.declare_dram_parameter("wo", [IN_DIM, IN_DIM], BF16, isOutput=False)
    bias_in = nc.declare_dram_parameter("bqkv", [1, 3 * IN_DIM], BF16, isOutput=False)
    mask_in = nc.declare_dram_parameter("mask", [128, 128], F32, isOutput=False)
    y_out = nc.declare_dram_parameter("y", [S, IN_DIM], F32, isOutput=True)
    wf_out = nc.declare_dram_parameter("wf", [128, NPAIR, 128], F32, isOutput=True)

    xv = x_in.rearrange("(n c) i -> n c i", c=C)
    yv = y_out.rearrange("(n c) i -> n c i", c=C)

    with tile.TileContext(nc) as tc:
        with (
            tc.tile_pool(name="singles", bufs=1) as singles,
            tc.tile_pool(name="xy", bufs=3) as xy,
            tc.tile_pool(name="work", bufs=2) as work,
            tc.tile_pool(name="small", bufs=3) as small,
            tc.tile_pool(name="ampool", bufs=4) as ampool,
            tc.tile_pool(name="qkv_ps", bufs=2, space="PSUM") as qkv_ps,
            tc.tile_pool(name="attn_ps", bufs=2, space="PSUM") as attn_ps,
            tc.tile_pool(name="tp_ps", bufs=2, space="PSUM") as tp_ps,
        ):
            # ---- persistent tiles ----
            wq_sb = singles.tile([128, 8, IN_DIM], BF16, tag="wq")
            wk_sb = singles.tile([128, 8, IN_DIM], BF16, tag="wk")
            wv_sb = singles.tile([128, 8, IN_DIM], BF16, tag="wv")
            wo_sb = singles.tile([128, 8, IN_DIM], BF16, tag="wo")
            for sb, src in ((wq_sb, wq_in), (wk_sb, wk_in), (wv_sb, wv_in), (wo_sb, wo_in)):
                nc.gpsimd.dma_start(out=sb, in_=src.rearrange("(ko ki) n -> ki ko n", ki=128))

            bias_sb = singles.tile([1, 3 * IN_DIM], BF16, tag="bias")
            nc.gpsimd.dma_start(out=bias_sb, in_=bias_in[:])
            ones_sb = singles.tile([1, 128], BF16, tag="ones")
            nc.vector.memset(ones_sb, 1.0)
            mask_sb = singles.tile([128, 128], F32, tag="mask")
            nc.gpsimd.dma_start(out=mask_sb, in_=mask_in[:])
            eps_sb = singles.tile([128, 1], F32, tag="eps")
            nc.vector.memset(eps_sb, 1e-5)
            ident = singles.tile([128, 128], BF16, tag="ident")
            make_identity(nc, ident)

            def pe_transpose(dst, src, n_tiles):
                # src: SBUF bf16 [128, n*128]; dst: SBUF bf16 [128, n, 128]
                for g in range(0, n_tiles, 4):
                    gn = min(4, n_tiles - g)
                    tp = tp_ps.tile([128, 4, 128], BF16, tag="tp")
                    for j in range(gn):
                        nc.tensor.transpose(tp[:, j], src[:, 128 * (g + j): 128 * (g + j + 1)], ident)
                    nc.scalar.copy(out=dst[:, g:g + gn], in_=tp[:, :gn])

            # recurrent state, fp32 accumulator + bf16 mirror for matmul
            state_sb = singles.tile([128, NPAIR, 128], F32, tag="state")
            nc.gpsimd.dma_start(out=state_sb, in_=st_in[:])
            state_bf = singles.tile([128, NPAIR, 128], BF16, tag="statebf")
            nc.vector.tensor_copy(out=state_bf, in_=state_sb)

            for c in range(n_chunks):
                # ---- load x chunk ----
                x_t = xy.tile([128, IN_DIM], F32, tag="x")
                nc.gpsimd.dma_start(out=x_t, in_=xv[c])

                # ---- LayerNorm (gamma/beta folded into weights on host) ----
                stats = small.tile([128, 2, 6], F32, tag="stats")
                nc.vector.bn_stats(out=stats[:, 0], in_=x_t[:, 0:512])
                nc.vector.bn_stats(out=stats[:, 1], in_=x_t[:, 512:1024])
                mv = small.tile([128, 2], F32, tag="mv")
                nc.vector.bn_aggr(out=mv, in_=stats)
                rstd = small.tile([128, 1], F32, tag="rstd")
                nc.scalar.activation(out=rstd, in_=mv[:, 1:2], func=AF.Sqrt, bias=eps_sb)
                nc.vector.reciprocal(out=rstd, in_=rstd)
                h_t = work.tile([128, IN_DIM], BF16, tag="h")
                nc.vector.tensor_scalar(out=h_t, in0=x_t, scalar1=mv[:, 0:1],
                                        scalar2=rstd, op0=ALU.subtract, op1=ALU.mult)

                # ---- transpose h (feature-major for matmul contraction) ----
                hT = work.tile([128, 8, 128], BF16, tag="hT")
                pe_transpose(hT, h_t, 8)

                # ---- qkv matmuls (natural layout: tokens on partitions) ----
                def qkv_matmul(w_sb, bias_off, tag):
                    ps = qkv_ps.tile([128, IN_DIM], F32, tag="qkvps")
                    for n in range(2):
                        sl = slice(n * 512, (n + 1) * 512)
                        nc.tensor.matmul(ps[:, sl], lhsT=ones_sb,
                                         rhs=bias_sb[:, bias_off + n * 512: bias_off + (n + 1) * 512],
                                         start=True, stop=False)
                        for ki in range(8):
                            nc.tensor.matmul(ps[:, sl], lhsT=hT[:, ki], rhs=w_sb[:, ki, sl],
                                             start=False, stop=(ki == 7))
                    return ps

                q_ps = qkv_matmul(wq_sb, 0, "q")
                k_ps = qkv_matmul(wk_sb, IN_DIM, "k")
                v_ps = qkv_matmul(wv_sb, 2 * IN_DIM, "v")

                # ---- feature map: elu(x)+1 = min(exp(x),1) + relu(x), then /(sum+eps)
                def elu_norm(ps, tag):
                    ex = work.tile([128, IN_DIM], F32, tag="ex")
                    nc.scalar.activation(out=ex, in_=ps, func=AF.Exp)
                    rl = work.tile([128, IN_DIM], F32, tag="rl")
                    nc.scalar.activation(out=rl, in_=ps, func=AF.Relu)
                    el = work.tile([128, NUM_HEAD, DIM_HEAD], F32, tag="el")
                    el_flat = el.rearrange("p h d -> p (h d)")
                    nc.vector.scalar_tensor_tensor(out=el_flat, in0=ex, scalar=1.0,
                                                   in1=rl, op0=ALU.min, op1=ALU.add)
                    s = small.tile([128, NUM_HEAD], F32, tag="s" + tag)
                    nc.vector.tensor_reduce(out=s, in_=el, axis=mybir.AxisListType.X, op=ALU.add)
                    nc.vector.tensor_scalar_add(out=s, in0=s, scalar1=1e-5)
                    nc.vector.reciprocal(out=s, in_=s)
                    nrm = work.tile([128, NUM_HEAD, DIM_HEAD], BF16, tag="n" + tag)
                    nc.vector.tensor_tensor(out=nrm, in0=el,
                                            in1=s[:, :, None].to_broadcast(el.shape),
                                            op=ALU.mult)
                    return nrm

                qn = elu_norm(q_ps, "q")
                kn = elu_norm(k_ps, "k")
                vn = work.tile([128, NUM_HEAD, DIM_HEAD], BF16, tag="vn")
                nc.scalar.copy(out=vn.rearrange("p h d -> p (h d)"), in_=v_ps)

                # ---- transpose q,k (head-dim on partitions) ----
                qnT = work.tile([128, NPAIR, 128], BF16, tag="qnT")
                knT = work.tile([128, NPAIR, 128], BF16, tag="knT")
                pe_transpose(qnT, qn.rearrange("p h d -> p (h d)"), 8)
                pe_transpose(knT, kn.rearrange("p h d -> p (h d)"), 8)

                # ---- per-head chunked attention ----
                oT = work.tile([128, NPAIR, 128], BF16, tag="oT")
                for h in range(NUM_HEAD):
                    p, half = h // 2, h % 2
                    prt = slice(64 * half, 64 * half + 64)
                    at = attn_ps.tile([128, 3, 128], F32, tag="at")
                    # A^T[kt, qt] = sum_d k[kt,d] q[qt,d]
                    nc.tensor.matmul(at[:, 0], lhsT=knT[prt, p], rhs=qnT[prt, p],
                                     start=True, stop=True)
                    am = ampool.tile([128, 128], BF16, tag="am")
                    nc.vector.tensor_tensor(out=am, in0=at[:, 0], in1=mask_sb, op=ALU.mult)
                    # O^T[e, qt] = sum_d W0[d,e] q[qt,d]  +  sum_kt v[kt,e] Amask[kt,qt]
                    nc.tensor.matmul(at[0:64, 1], lhsT=state_bf[prt, p, prt],
                                     rhs=qnT[prt, p], start=True, stop=False)
                    nc.tensor.matmul(at[0:64, 1], lhsT=vn[:, h], rhs=am,
                                     start=False, stop=True)
                    nc.scalar.copy(out=oT[prt, p], in_=at[0:64, 1])
                    # state update: W0 += K^T V
                    nc.tensor.matmul(at[0:64, 2, 0:64], lhsT=kn[:, h], rhs=vn[:, h],
                                     start=True, stop=True)
                    nc.vector.tensor_add(out=state_sb[prt, p, prt],
                                         in0=state_sb[prt, p, prt], in1=at[0:64, 2, 0:64])
                    nc.scalar.copy(out=state_bf[prt, p, prt], in_=state_sb[prt, p, prt])

                # ---- out matmul + residual ----
                y_t = xy.tile([128, IN_DIM], F32, tag="y")
                for n in range(2):
                    sl = slice(n * 512, (n + 1) * 512)
                    ops = qkv_ps.tile([128, 1024], F32, tag="qkvps", name="ops")[:, :512]
                    for p8 in range(8):
                        nc.tensor.matmul(ops, lhsT=oT[:, p8], rhs=wo_sb[:, p8, sl],
                                         start=(p8 == 0), stop=(p8 == 7))
                    nc.vector.tensor_add(out=y_t[:, sl], in0=ops, in1=x_t[:, sl])
                nc.gpsimd.dma_start(out=yv[c], in_=y_t)

            # ---- final state out ----
            nc.gpsimd.dma_start(out=wf_out[:], in_=state_sb)

    if split_waits:
        _split_excess_waits(nc)
    return nc


# ---------------- host side ----------------

def _prep_weights(W_qkv, W_out, ln_gamma, ln_beta):
    H, D3 = NUM_HEAD, 3 * DIM_HEAD
    W_eff = (W_qkv * ln_gamma[None, :]).astype(np.float32)
    bias_full = W_qkv.astype(np.float64) @ ln_beta.astype(np.float64)  # [3072]
    Wr = W_eff.reshape(H, D3, IN_DIM)
    bq = bias_full.reshape(H, D3)
    wq = Wr[:, 0:DIM_HEAD].reshape(H * DIM_HEAD, IN_DIM)
    wk = Wr[:, DIM_HEAD:2 * DIM_HEAD].reshape(H * DIM_HEAD, IN_DIM)
    wv = Wr[:, 2 * DIM_HEAD:].reshape(H * DIM_HEAD, IN_DIM)
    bias = np.concatenate([bq[:, 0:DIM_HEAD].reshape(-1),
                           bq[:, DIM_HEAD:2 * DIM_HEAD].reshape(-1),
                           bq[:, 2 * DIM_HEAD:].reshape(-1)])[None, :]
    bf = ml_dtypes.bfloat16
    return (np.ascontiguousarray(wq.T).astype(bf),
            np.ascontiguousarray(wk.T).astype(bf),
            np.ascontiguousarray(wv.T).astype(bf),
            np.ascontiguousarray(W_out.T).astype(bf),
            bias.astype(bf))


def _pack_state(state_b):
    out = np.zeros((128, NPAIR, 128), np.float32)
    for p in range(NPAIR):
        out[0:64, p, 0:64] = state_b[2 * p]
        out[64:128, p, 64:128] = state_b[2 * p + 1]
    return out


def _unpack_state(wf_dev):
    out = np.empty((NUM_HEAD, DIM_HEAD, DIM_HEAD), np.float32)
    for p in range(NPAIR):
        out[2 * p] = wf_dev[0:64, p, 0:64]
        out[2 * p + 1] = wf_dev[64:128, p, 64:128]
    return out


_NC_CACHE = {}
TRACE = False
LAST_RESULT = None


def kernel(x, state, W_qkv, W_out, ln_gamma, ln_beta):
    x = np.asarray(x, np.float32)
    state = np.asarray(state, np.float32)
    S, B, _ = x.shape
    n_chunks = S // C
    if n_chunks not in _NC_CACHE:
        _NC_CACHE[n_chunks] = build_nc(n_chunks)
    nc = _NC_CACHE[n_chunks]

    wq, wk, wv, wo, bias = _prep_weights(np.asarray(W_qkv, np.float32),
                                         np.asarray(W_out, np.float32),
                                         np.asarray(ln_gamma, np.float32),
                                         np.asarray(ln_beta, np.float32))
    mask = np.triu(np.ones((128, 128), np.float32))

    in_maps = []
    for b in range(B):
        in_maps.append({
            "x": np.ascontiguousarray(x[:, b, :]),
            "st": _pack_state(state[b]),
            "wq": wq, "wk": wk, "wv": wv, "wo": wo,
            "bqkv": bias, "mask": mask,
        })
    global LAST_RESULT
    res = run_bass_kernel_spmd(nc, in_maps, core_ids=list(range(B)), trace=TRACE)
    LAST_RESULT = res

    y = np.empty((S, B, IN_DIM), np.float32)
    wf = np.empty((B, NUM_HEAD, DIM_HEAD, DIM_HEAD), np.float32)
    for b in range(B):
        y[:, b, :] = res.results[b]["y"]
        wf[b] = _unpack_state(res.results[b]["wf"])
    return y, wf


# revision 38
# speedup vs baseline: 1.1054x; 1.1054x over previous
"""Additive fast-weight (linear attention) layer on 8 TRN2 NeuronCores.

Strategy:
  - Shard batch (B=8) across the 8 cores: the recurrent state and scan are
    independent per (b, h); slow-net / out-linear weights are replicated.
  - Per core: LayerNorm -> qkv matmul -> elu+1 feature map with sum-norm ->
    chunked-parallel linear attention (chunk C=128 converts the 2048-step
    scan into matmuls: O = Q.W0 + tril(Q K^T).V ; W0 += K^T V) -> out matmul
    -> residual.
  - Matmuls run in bf16 with fp32 PSUM accumulation; the recurrent state is
    accumulated in fp32 in SBUF.  LayerNorm gamma is folded into W_qkv and
    beta into a per-feature bias (added via a K=1 matmul) on the host.
"""

import os
import numpy as np
import ml_dtypes

import concourse.bass as bass
import concourse.mybir as mybir
import concourse.tile as tile
from concourse.bass_utils import run_bass_kernel_spmd
from concourse.masks import make_identity

F32 = mybir.dt.float32
BF16 = mybir.dt.bfloat16
AF = mybir.ActivationFunctionType
ALU = mybir.AluOpType

NUM_HEAD, DIM_HEAD, IN_DIM = 16, 64, 1024
SLEN, BSZ = 2048, 8
C = 128  # chunk length (tokens per chunk)
NPAIR = NUM_HEAD // 2  # head pairs (2 heads share a 128-partition tile)


# Max semaphore waits each ISA struct can encode (observed from walrus
# codegen failures / successes).  Excess waits are moved onto standalone
# EventSemaphore instructions on the same engine right before, which is
# sound for compute engines (sequencer executes in program order).
_WAIT_CAPS = {}
_DMA_TYPES = {"InstDMACopy", "InstDmaTransposeAnt"}
_NO_SPLIT = {
    "InstEventSemaphore", "InstCall", "InstUnconditionalBranch",
    "InstRegisterMove", "InstISA",
}


def _split_excess_waits(nc):
    """Walrus can encode only one sem-wait per instruction (per ISA struct).

    Compute engines: move excess waits onto standalone EventSemaphore
    instructions on the same engine right before (program order makes this
    sound).  DMA descriptors: the wait fires the descriptor, so instead a
    chain of EventSemaphores on SP absorbs ALL original waits and then
    increments a fresh one-shot semaphore that the DMA's single slot waits on.
    """
    used_ids = set()
    for f in nc.m.functions:
        for blk in f.blocks:
            for inst in blk.instructions:
                si = inst.sync_info
                if si is not None:
                    for w in (si.on_wait or []):
                        if w.sync_type == 'semaphore':
                            used_ids.add(w.id)
                    for u in (si.on_update or []):
                        if u.sync_type == 'semaphore':
                            used_ids.add(u.id)
    free_ids = sorted([i for i in range(256) if i not in used_ids], reverse=True)  # pop() -> lowest first
    sink_id = free_ids.pop()

    def _sink_upd():
        return [mybir.SyncUpdate(sync_type='semaphore', id=sink_id,
                                 ant_name='wsink', update_mode='sem-inc',
                                 update_value=1, update_reg=None)]

    n_split = 0
    n_join = 0
    for f in nc.m.functions:
        for blk in f.blocks:
            out = []
            changed = False
            for inst in blk.instructions:
                si = inst.sync_info
                tn = type(inst).__name__
                if si is not None and si.on_wait and tn not in _NO_SPLIT:
                    waits = list(si.on_wait)
                    cap = _WAIT_CAPS.get(tn, 1)
                    is_dma = tn in _DMA_TYPES and inst.engine != mybir.EngineType.Pool
                    if is_dma and len(waits) > 1 and os.environ.get('BISECT_DROP'):
                        inst.sync_info = mybir.SyncInfo(
                            on_wait=[waits[0]], on_update=list(si.on_update))
                        changed = True
                    elif is_dma and len(waits) > 1 and not os.environ.get('NO_JOIN'):
                        sem_id = free_ids.pop()
                        sem_name = f"wjoin{n_join}"
                        n_join += 1
                        for i, w in enumerate(waits):
                            ev = mybir.InstEventSemaphore(
                                name=f"I-wjoin-{n_join}-{i}", ins=[], outs=[])
                            ev.engine = mybir.EngineType.Pool
                            upd = _sink_upd()
                            if i == len(waits) - 1:
                                upd = [mybir.SyncUpdate(
                                    sync_type='semaphore', id=sem_id,
                                    ant_name=sem_name, update_mode='sem-inc',
                                    update_value=1, update_reg=None)]
                            ev.sync_info = mybir.SyncInfo(on_wait=[w], on_update=upd)
                            out.append(ev)
                        inst.sync_info = mybir.SyncInfo(
                            on_wait=[mybir.SyncWait(
                                sync_type='semaphore', id=sem_id,
                                ant_name=sem_name, wait_mode='sem-ge-imm',
                                wait_value=1, wait_reg=None)],
                            on_update=list(si.on_update))
                        changed = True
                    elif not is_dma and len(waits) > cap:
                        for w in waits[cap:]:
                            ev = mybir.InstEventSemaphore(
                                name=f"I-wsplit-{n_split}", ins=[], outs=[])
                            n_split += 1
                            ev.engine = inst.engine
                            ev.sync_info = mybir.SyncInfo(on_wait=[w], on_update=_sink_upd())
                            out.append(ev)
                        inst.sync_info = mybir.SyncInfo(
                            on_wait=waits[:cap], on_update=list(si.on_update))
                        changed = True
                out.append(inst)
            if changed:
                blk.instructions = out
    if n_join or n_split:
        reg = dict(nc.m.ant_sem_names)
        reg[str(sink_id)] = ['wsink']
        for f in nc.m.functions:
            for blk in f.blocks:
                for inst in blk.instructions:
                    si = inst.sync_info
                    if si is not None:
                        for u in (si.on_update or []):
                            if u.ant_name.startswith("wjoin"):
                                reg[str(u.id)] = [u.ant_name]
        nc.m.ant_sem_names = reg
    return n_split + n_join


def build_nc(n_chunks: int = SLEN // C, split_waits: bool = True) -> bass.Bass:
    S = n_chunks * C
    nc = bass.Bass()

    x_in = nc.declare_dram_parameter("x", [S, IN_DIM], F32, isOutput=False)
    st_in = nc.declare_dram_parameter("st", [128, NPAIR, 128], F32, isOutput=False)
    wq_in = nc.declare_dram_parameter("wq", [IN_DIM, IN_DIM], BF16, isOutput=False)
    wk_in = nc.declare_dram_parameter("wk", [IN_DIM, IN_DIM], BF16, isOutput=False)
    wv_in = nc.declare_dram_parameter("wv", [IN_DIM, IN_DIM], BF16, isOutput=False)
    wo_in = nc.declare_dram_parameter("wo", [IN_DIM, IN_DIM], BF16, isOutput=False)
    bias_in = nc.declare_dram_parameter("bqkv", [1, 3 * IN_DIM], BF16, isOutput=False)
    mask_in = nc.declare_dram_parameter("mask", [128, 128], F32, isOutput=False)
    y_out = nc.declare_dram_parameter("y", [S, IN_DIM], F32, isOutput=True)
    wf_out = nc.declare_dram_parameter("wf", [128, NPAIR, 128], F32, isOutput=True)

    xv = x_in.rearrange("(n c) i -> n c i", c=C)
    yv = y_out.rearrange("(n c) i -> n c i", c=C)

    with tile.TileContext(nc) as tc:
        with (
            tc.tile_pool(name="singles", bufs=1) as singles,
            tc.tile_pool(name="xy", bufs=3) as xy,
            tc.tile_pool(name="work", bufs=2) as work,
            tc.tile_pool(name="small", bufs=3) as small,
            tc.tile_pool(name="ampool", bufs=4) as ampool,
            tc.tile_pool(name="qkv_ps", bufs=3, space="PSUM") as qkv_ps,
            tc.tile_pool(name="attn_ps", bufs=3, space="PSUM") as attn_ps,
            tc.tile_pool(name="tp_ps", bufs=2, space="PSUM") as tp_ps,
        ):
            # ---- persistent tiles ----
            wq_sb = singles.tile([128, 8, IN_DIM], BF16, tag="wq")
            wk_sb = singles.tile([128, 8, IN_DIM], BF16, tag="wk")
            wv_sb = singles.tile([128, 8, IN_DIM], BF16, tag="wv")
            wo_sb = singles.tile([128, 8, IN_DIM], BF16, tag="wo")
            for sb, src in ((wq_sb, wq_in), (wk_sb, wk_in), (wv_sb, wv_in), (wo_sb, wo_in)):
                nc.gpsimd.dma_start(out=sb, in_=src.rearrange("(ko ki) n -> ki ko n", ki=128))

            bias_sb = singles.tile([1, 3 * IN_DIM], BF16, tag="bias")
            nc.gpsimd.dma_start(out=bias_sb, in_=bias_in[:])
            ones_sb = singles.tile([1, 128], BF16, tag="ones")
            nc.vector.memset(ones_sb, 1.0)
            mask_sb = singles.tile([128, 128], F32, tag="mask")
            nc.gpsimd.dma_start(out=mask_sb, in_=mask_in[:])
            eps_sb = singles.tile([128, 1], F32, tag="eps")
            nc.vector.memset(eps_sb, 1e-5)
            ident = singles.tile([128, 128], BF16, tag="ident")
            make_identity(nc, ident)

            def pe_transpose(dst, src, n_tiles):
                # src: SBUF bf16 [128, n*128]; dst: SBUF bf16 [128, n, 128]
                for g in range(0, n_tiles, 4):
                    gn = min(4, n_tiles - g)
                    tp = tp_ps.tile([128, 4, 128], BF16, tag="tp")
                    for j in range(gn):
                        nc.tensor.transpose(tp[:, j], src[:, 128 * (g + j): 128 * (g + j + 1)], ident)
                    nc.scalar.copy(out=dst[:, g:g + gn], in_=tp[:, :gn])

            # recurrent state, fp32 accumulator + bf16 mirror for matmul
            state_sb = singles.tile([128, NPAIR, 128], F32, tag="state")
            nc.gpsimd.dma_start(out=state_sb, in_=st_in[:])
            state_bf = singles.tile([128, NPAIR, 128], BF16, tag="statebf")
            nc.vector.tensor_copy(out=state_bf, in_=state_sb)

            for c in range(n_chunks):
                # ---- load x chunk ----
                x_t = xy.tile([128, IN_DIM], F32, tag="x")
                nc.gpsimd.dma_start(out=x_t, in_=xv[c])

                # ---- LayerNorm (gamma/beta folded into weights on host) ----
                stats = small.tile([128, 2, 6], F32, tag="stats")
                nc.vector.bn_stats(out=stats[:, 0], in_=x_t[:, 0:512])
                nc.vector.bn_stats(out=stats[:, 1], in_=x_t[:, 512:1024])
                mv = small.tile([128, 2], F32, tag="mv")
                nc.vector.bn_aggr(out=mv, in_=stats)
                rstd = small.tile([128, 1], F32, tag="rstd")
                nc.scalar.activation(out=rstd, in_=mv[:, 1:2], func=AF.Sqrt, bias=eps_sb)
                nc.vector.reciprocal(out=rstd, in_=rstd)
                h_t = work.tile([128, IN_DIM], BF16, tag="h")
                nc.vector.tensor_scalar(out=h_t, in0=x_t, scalar1=mv[:, 0:1],
                                        scalar2=rstd, op0=ALU.subtract, op1=ALU.mult)

                # ---- transpose h (feature-major for matmul contraction) ----
                hT = work.tile([128, 8, 128], BF16, tag="hT")
                pe_transpose(hT, h_t, 8)

                # ---- qkv matmuls (natural layout: tokens on partitions) ----
                def qkv_matmul_half(w_sb, bias_off, n):
                    ps = qkv_ps.tile([128, 512], F32, tag="qkvps", name="qkvps")
                    sl = slice(n * 512, (n + 1) * 512)
                    nc.tensor.matmul(ps, lhsT=ones_sb,
                                     rhs=bias_sb[:, bias_off + n * 512: bias_off + (n + 1) * 512],
                                     start=True, stop=False)
                    for ki in range(8):
                        nc.tensor.matmul(ps, lhsT=hT[:, ki], rhs=w_sb[:, ki, sl],
                                         start=False, stop=(ki == 7))
                    return ps

                q_ps = [qkv_matmul_half(wq_sb, 0, n) for n in range(2)]
                k_ps = [qkv_matmul_half(wk_sb, IN_DIM, n) for n in range(2)]
                v_ps = [qkv_matmul_half(wv_sb, 2 * IN_DIM, n) for n in range(2)]

                # ---- feature map: elu(x)+1 = min(exp(x),1) + relu(x), then /(sum+eps)
                def elu_norm(ps_halves, tag):
                    nrm = work.tile([128, NUM_HEAD, DIM_HEAD], BF16, tag="n" + tag)
                    for n, ps in enumerate(ps_halves):
                        hh = slice(8 * n, 8 * n + 8)  # heads in this half
                        ex = work.tile([128, 512], F32, tag="ex", name="ex")
                        nc.scalar.activation(out=ex, in_=ps, func=AF.Exp)
                        rl = work.tile([128, 512], F32, tag="rl", name="rl")
                        nc.scalar.activation(out=rl, in_=ps, func=AF.Relu)
                        el = work.tile([128, 8, DIM_HEAD], F32, tag="el", name="el")
                        nc.vector.scalar_tensor_tensor(out=el.rearrange("p h d -> p (h d)"),
                                                       in0=ex, scalar=1.0,
                                                       in1=rl, op0=ALU.min, op1=ALU.add)
                        sm = small.tile([128, 8], F32, tag="s" + tag, name="sm")
                        nc.vector.tensor_reduce(out=sm, in_=el, axis=mybir.AxisListType.X, op=ALU.add)
                        nc.vector.tensor_scalar_add(out=sm, in0=sm, scalar1=1e-5)
                        nc.vector.reciprocal(out=sm, in_=sm)
                        nc.vector.tensor_tensor(out=nrm[:, hh], in0=el,
                                                in1=sm[:, :, None].to_broadcast(el.shape),
                                                op=ALU.mult)
                    return nrm

                qn = elu_norm(q_ps, "q")
                kn = elu_norm(k_ps, "k")
                vn = work.tile([128, NUM_HEAD, DIM_HEAD], BF16, tag="vn")
                for n in range(2):
                    nc.scalar.copy(out=vn[:, 8 * n: 8 * n + 8].rearrange("p h d -> p (h d)"),
                                   in_=v_ps[n])

                # ---- transpose q,k (head-dim on partitions) ----
                qnT = work.tile([128, NPAIR, 128], BF16, tag="qnT")
                knT = work.tile([128, NPAIR, 128], BF16, tag="knT")
                pe_transpose(qnT, qn.rearrange("p h d -> p (h d)"), 8)
                pe_transpose(knT, kn.rearrange("p h d -> p (h d)"), 8)

                # ---- per-head chunked attention ----
                oT = work.tile([128, NPAIR, 128], BF16, tag="oT")
                for h in range(NUM_HEAD):
                    p, half = h // 2, h % 2
                    prt = slice(64 * half, 64 * half + 64)
                    at = attn_ps.tile([128, 3, 128], F32, tag="at")
                    nc.tensor.matmul(at[:, 0], lhsT=knT[prt, p], rhs=qnT[prt, p],
                                     start=True, stop=True)
                    am = ampool.tile([128, 128], BF16, tag="am")
                    nc.vector.tensor_tensor(out=am, in0=at[:, 0], in1=mask_sb, op=ALU.mult)
                    nc.tensor.matmul(at[0:64, 1], lhsT=state_bf[prt, p, prt],
                                     rhs=qnT[prt, p], start=True, stop=False)
                    nc.tensor.matmul(at[0:64, 1], lhsT=vn[:, h], rhs=am,
                                     start=False, stop=True)
                    nc.scalar.copy(out=oT[prt, p], in_=at[0:64, 1])
                    nc.tensor.matmul(at[0:64, 2, 0:64], lhsT=kn[:, h], rhs=vn[:, h],
                                     start=True, stop=True)
                    nc.vector.tensor_add(out=state_sb[prt, p, prt],
                                         in0=state_sb[prt, p, prt], in1=at[0:64, 2, 0:64])
                    nc.scalar.copy(out=state_bf[prt, p, prt], in_=state_sb[prt, p, prt])

                # ---- out matmul + residual ----
                y_t = xy.tile([128, IN_DIM], F32, tag="y")
                for n in range(2):
                    sl = slice(n * 512, (n + 1) * 512)
                    ops = qkv_ps.tile([128, 512], F32, tag="qkvps", name="ops")
                    for p8 in range(8):
                        nc.tensor.matmul(ops, lhsT=oT[:, p8], rhs=wo_sb[:, p8, sl],
                                         start=(p8 == 0), stop=(p8 == 7))
                    nc.vector.tensor_add(out=y_t[:, sl], in0=ops, in1=x_t[:, sl])
                nc.gpsimd.dma_start(out=yv[c], in_=y_t)

            # ---- final state out ----
            nc.gpsimd.dma_start(out=wf_out[:], in_=state_sb)

    if split_waits:
        _split_excess_waits(nc)
    return nc


# ---------------- host side ----------------

def _prep_weights(W_qkv, W_out, ln_gamma, ln_beta):
    H, D3 = NUM_HEAD, 3 * DIM_HEAD
    W_eff = (W_qkv * ln_gamma[None, :]).astype(np.float32)
    bias_full = W_qkv.astype(np.float64) @ ln_beta.astype(np.float64)  # [3072]
    Wr = W_eff.reshape(H, D3, IN_DIM)
    bq = bias_full.reshape(H, D3)
    wq = Wr[:, 0:DIM_HEAD].reshape(H * DIM_HEAD, IN_DIM)
    wk = Wr[:, DIM_HEAD:2 * DIM_HEAD].reshape(H * DIM_HEAD, IN_DIM)
    wv = Wr[:, 2 * DIM_HEAD:].reshape(H * DIM_HEAD, IN_DIM)
    bias = np.concatenate([bq[:, 0:DIM_HEAD].reshape(-1),
                           bq[:, DIM_HEAD:2 * DIM_HEAD].reshape(-1),
                           bq[:, 2 * DIM_HEAD:].reshape(-1)])[None, :]
    bf = ml_dtypes.bfloat16
    return (np.ascontiguousarray(wq.T).astype(bf),
            np.ascontiguousarray(wk.T).astype(bf),
            np.ascontiguousarray(wv.T).astype(bf),
            np.ascontiguousarray(W_out.T).astype(bf),
            bias.astype(bf))


def _pack_state(state_b):
    out = np.zeros((128, NPAIR, 128), np.float32)
    for p in range(NPAIR):
        out[0:64, p, 0:64] = state_b[2 * p]
        out[64:128, p, 64:128] = state_b[2 * p + 1]
    return out


def _unpack_state(wf_dev):
    out = np.empty((NUM_HEAD, DIM_HEAD, DIM_HEAD), np.float32)
    for p in range(NPAIR):
        out[2 * p] = wf_dev[0:64, p, 0:64]
        out[2 * p + 1] = wf_dev[64:128, p, 64:128]
    return out


_NC_CACHE = {}
TRACE = False
LAST_RESULT = None


def kernel(x, state, W_qkv, W_out, ln_gamma, ln_beta):
    x = np.asarray(x, np.float32)
    state = np.asarray(state, np.float32)
    S, B, _ = x.shape
    n_chunks = S // C
    if n_chunks not in _NC_CACHE:
        _NC_CACHE[n_chunks] = build_nc(n_chunks)
    nc = _NC_CACHE[n_chunks]

    wq, wk, wv, wo, bias = _prep_weights(np.asarray(W_qkv, np.float32),
                                         np.asarray(W_out, np.float32),
                                         np.asarray(ln_gamma, np.float32),
                                         np.asarray(ln_beta, np.float32))
    mask = np.triu(np.ones((128, 128), np.float32))

    in_maps = []
    for b in range(B):
        in_maps.append({
            "x": np.ascontiguousarray(x[:, b, :]),
            "st": _pack_state(state[b]),
            "wq": wq, "wk": wk, "wv": wv, "wo": wo,
            "bqkv": bias, "mask": mask,
        })
    global LAST_RESULT
    res = run_bass_kernel_spmd(nc, in_maps, core_ids=list(range(B)), trace=TRACE)
    LAST_RESULT = res

    y = np.empty((S, B, IN_DIM), np.float32)
    wf = np.empty((B, NUM_HEAD, DIM_HEAD, DIM_HEAD), np.float32)
    for b in range(B):
        y[:, b, :] = res.results[b]["y"]
        wf[b] = _unpack_state(res.results[b]["wf"])
    return y, wf
